# revision 7
# baseline (speedup 1.0000x reference)
"""LoRALinear kernel for Trainium2 (8 NeuronCores, SPMD data-parallel).

Computes out = x @ W.T + b + SCALE*((x@gA.T)@gB.T + (x@lA.T)@lB.T)
  x: [8, 2048, 1024] f32, W: [4096, 1024], b: [4096]
  gA/lA: [8, 1024], gB/lB: [4096, 8]  ->  out: [8, 2048, 4096] f32

Data-parallel: core i handles batch i. Host marshals layouts so the
device does nothing but matmuls and psum evictions:
  - xT   [1024, 2048] fp16: x[i].T  (k on partitions -> no PE transposes)
  - WtT  [8192, 512]  fp16: W.T tiled [ot][kt][128, 512] so o-tile ot is
    one contiguous 1MB chunk (ot-outer pipeline starts after 1MB of DMA)
  - A_cat = SCALE*[gA;lA] [16, 1024], B_catT = [gB.T;lB.T] [16, 4096]

Device, per o-tile (512 cols), software-pipelined one ahead:
  build W_eff chunk: DMA W.T chunk + rank-16 LoRA matmul into f32 psum,
  DVE-added in place (fp16).  Then 16 s-tiles x 8 k-tile fp16 matmuls
  accumulate into f32 psum; DVE adds bias (PE-broadcast once) and writes
  fp16 out tile; DMA to DRAM. Host casts fp16 out back to f32.

All-fp16 PE ops keep LDWEIGHTS pipelined: main GEMM streams at
512 cols/matmul back-to-back = the 78.6 TF/s fp16 roofline.
fp16 in/out rounding gives ~8e-4 absmax rel err (f32 psum accumulate).
"""
import numpy as np
from contextlib import ExitStack

import concourse.bass as bass
import concourse.tile as tile
from concourse import bacc, mybir
from concourse.bass import ts, ds
from concourse.bass_utils import run_bass_kernel_spmd

F32 = mybir.dt.float32
F16 = mybir.dt.float16

N_CORES = 8
B, S, DIN, DOUT, R = 8, 2048, 1024, 4096, 8
SCALE = 16.0 / 8
R2 = 2 * R

P = 128            # partition tile
OTILE = 512        # matmul moving free dim (one PSUM bank of f32)
KT = DIN // P      # 8 k-tiles
OT = DOUT // OTILE # 8 o-tiles
ST = S // P        # 16 s-tiles


def build_nc():
    nc = bacc.Bacc("TRN2", target_bir_lowering=False, debug=False,
                   num_devices=N_CORES)
    xT = nc.dram_tensor("xT", [DIN, S], F16, kind="ExternalInput").ap()
    WtT = nc.dram_tensor("WtT", [OT * KT * P, OTILE], F16,
                         kind="ExternalInput").ap()
    bvec = nc.dram_tensor("b16", [DOUT], F16, kind="ExternalInput").ap()
    A_cat = nc.dram_tensor("A_cat", [R2, DIN], F16, kind="ExternalInput").ap()
    B_catT = nc.dram_tensor("B_catT", [R2, DOUT], F16,
                            kind="ExternalInput").ap()
    out = nc.dram_tensor("out", [S, DOUT], F16, kind="ExternalOutput").ap()

    with tile.TileContext(nc) as tc:
        with ExitStack() as ctx:
            const = ctx.enter_context(tc.tile_pool(name="const", bufs=1))
            xt_pool = ctx.enter_context(tc.tile_pool(name="xt", bufs=1))
            wet_pool = ctx.enter_context(tc.tile_pool(name="wet", bufs=3))
            out_pool = ctx.enter_context(tc.tile_pool(name="outp", bufs=4))
            ps_aux = ctx.enter_context(
                tc.tile_pool(name="psaux", bufs=4, space="PSUM"))
            ps_main = ctx.enter_context(
                tc.tile_pool(name="psmain", bufs=4, space="PSUM"))

            # consts; sync queue gets [acat, bcatt, wet...] triggers,
            # scalar queue gets [brow, xt..., out...] triggers (each
            # dma_start costs ~600ns serialized on its trigger queue)
            ones_col = const.tile([1, P], F16)
            nc.vector.memset(ones_col[:], 1.0)
            acat = const.tile([R2, DIN], F16)
            nc.sync.dma_start(acat[:], A_cat)
            bcatt = const.tile([R2, DOUT], F16)
            nc.sync.dma_start(bcatt[:], B_catT)
            brow16 = const.tile([1, DOUT], F16)
            nc.scalar.dma_start(brow16[:], bvec[None, :])
            bias_sb = const.tile([P, DOUT], F32)

            # PE p-state warmup: the PE clock ramps 0.65->1.2->2.4 GHz over
            # ~3us of continuous work; run short dummy matmuls (dep only on
            # the memset) so the real stream starts at full clock. Sized to
            # end right as the first LoRA matmul's operands land (~11us).
            for i in range(52):
                pw = ps_aux.tile([P, OTILE], F32, tag="psaux")
                nc.tensor.matmul(pw[:, :P], ones_col[:], ones_col[:],
                                 start=True, stop=True)

            # W_eff chunks, triple-buffered per kt tag: [128 k, 512 o] fp16
            wet = [[None] * KT for _ in range(OT)]

            def build_wet(ot):
                for kt in range(KT):
                    w = wet_pool.tile([P, OTILE], F16, tag=f"wet{kt}",
                                      name=f"wet{ot}_{kt}")
                    nc.sync.dma_start(
                        w[:], WtT[ds((ot * KT + kt) * P, P), :])
                    wet[ot][kt] = w
                for kt in range(KT):
                    pl = ps_aux.tile([P, OTILE], F32, tag="psaux")
                    nc.tensor.matmul(pl[:], acat[:, ts(kt, P)],
                                     bcatt[:, ts(ot, OTILE)],
                                     start=True, stop=True)
                    w = wet[ot][kt]
                    nc.vector.tensor_tensor(w[:], pl[:], w[:],
                                            mybir.AluOpType.add)

            build_wet(0)

            # resident x.T: 8 tiles [128 k, 2048 s], 4KB/partition
            xt = []
            for kt in range(KT):
                t = xt_pool.tile([P, S], F16, tag=f"xt{kt}", name=f"xt{kt}")
                nc.scalar.dma_start(t[:], xT[ts(kt, P), :])
                xt.append(t)

            build_wet(1)

            # bias broadcast to 128 partitions via rank-1 fp16 matmuls
            for ot in range(OT):
                pb = ps_aux.tile([P, OTILE], F32, tag="psaux")
                nc.tensor.matmul(pb[:], ones_col[:],
                                 brow16[:, ts(ot, OTILE)],
                                 start=True, stop=True)
                nc.vector.tensor_copy(bias_sb[:, ts(ot, OTILE)], pb[:])

            # ---- main: ot-outer; W_eff[ot+1] DMAs fire at s-loop start and
            # its 8 LoRA matmul+add pairs are woven between the first 8
            # s-tile groups (keeps psaux/DVE slack, no boundary stall) ----
            for ot in range(OT):
                nxt = ot + 1
                weave = 2 <= nxt < OT
                if weave:
                    for kt in range(KT):
                        w = wet_pool.tile([P, OTILE], F16, tag=f"wet{kt}",
                                          name=f"wet{nxt}_{kt}")
                        nc.sync.dma_start(
                            w[:], WtT[ds((nxt * KT + kt) * P, P), :])
                        wet[nxt][kt] = w
                for st in range(ST):
                    po = ps_main.tile([P, OTILE], F32, tag="psmain")
                    for kt in range(KT):
                        nc.tensor.matmul(po[:], xt[kt][:, ts(st, P)],
                                         wet[ot][kt][:],
                                         start=(kt == 0), stop=(kt == KT - 1))
                    osb = out_pool.tile([P, OTILE], F16)
                    nc.vector.tensor_tensor(osb[:], po[:],
                                            bias_sb[:, ts(ot, OTILE)],
                                            mybir.AluOpType.add)
                    nc.scalar.dma_start(out[ts(st, P), ts(ot, OTILE)], osb[:])
                    if weave and st < KT:
                        pl = ps_aux.tile([P, OTILE], F32, tag="psaux")
                        nc.tensor.matmul(pl[:], acat[:, ts(st, P)],
                                         bcatt[:, ts(nxt, OTILE)],
                                         start=True, stop=True)
                        w = wet[nxt][st]
                        nc.vector.tensor_tensor(w[:], pl[:], w[:],
                                                mybir.AluOpType.add)

    nc.compile()
    return nc


_NC_CACHE = None


def _get_nc():
    global _NC_CACHE
    if _NC_CACHE is None:
        _NC_CACHE = build_nc()
    return _NC_CACHE


def make_in_maps(x, W, b, global_A, global_B, local_A, local_B):
    x = np.asarray(x, dtype=np.float32)
    W = np.asarray(W, dtype=np.float32)
    b = np.asarray(b, dtype=np.float32)
    # W.T tiled [ot][kt][128, 512] -> [8192, 512] so each o-tile is contiguous
    WtT = np.ascontiguousarray(
        W.T.reshape(KT, P, OT, OTILE).transpose(2, 0, 1, 3)
    ).reshape(OT * KT * P, OTILE).astype(np.float16)
    A_cat = np.ascontiguousarray(
        SCALE * np.concatenate([np.asarray(global_A), np.asarray(local_A)],
                               axis=0)).astype(np.float16)
    B_catT = np.ascontiguousarray(
        np.concatenate([np.asarray(global_B).T, np.asarray(local_B).T],
                       axis=0)).astype(np.float16)
    b16 = b.astype(np.float16)
    return [
        {"xT": np.ascontiguousarray(x[i].T).astype(np.float16),
         "WtT": WtT, "b16": b16, "A_cat": A_cat, "B_catT": B_catT}
        for i in range(N_CORES)
    ]


def kernel(x, W, b, global_A, global_B, local_A, local_B):
    nc = _get_nc()
    in_maps = make_in_maps(x, W, b, global_A, global_B, local_A, local_B)
    res = run_bass_kernel_spmd(nc, in_maps, list(range(N_CORES))).results
    return np.stack([res[i]["out"].astype(np.float32)
                     for i in range(N_CORES)], axis=0)


# revision 8
# speedup vs baseline: 1.0175x; 1.0175x over previous
"""LoRALinear kernel for Trainium2 (8 NeuronCores, SPMD data-parallel).

Computes out = x @ W.T + b + SCALE*((x@gA.T)@gB.T + (x@lA.T)@lB.T)
  x: [8, 2048, 1024] f32, W: [4096, 1024], b: [4096]
  gA/lA: [8, 1024], gB/lB: [4096, 8]  ->  out: [8, 2048, 4096] f32

Data-parallel: core i handles batch i. Host marshals layouts so the
device does nothing but matmuls and psum evictions:
  - xT   [1024, 2048] fp16: x[i].T  (k on partitions -> no PE transposes)
  - WtT  [8192, 512]  fp16: W.T tiled [ot][kt][128, 512] so o-tile ot is
    one contiguous 1MB chunk (ot-outer pipeline starts after 1MB of DMA)
  - A_cat = SCALE*[gA;lA] [16, 1024], B_catT = [gB.T;lB.T] [16, 4096]

Device, per o-tile (512 cols), software-pipelined one ahead:
  build W_eff chunk: DMA W.T chunk + rank-16 LoRA matmul into f32 psum,
  DVE-added in place (fp16).  Then 16 s-tiles x 8 k-tile fp16 matmuls
  accumulate into f32 psum; DVE adds bias (PE-broadcast once) and writes
  fp16 out tile; DMA to DRAM. Host casts fp16 out back to f32.

All-fp16 PE ops keep LDWEIGHTS pipelined: main GEMM streams at
512 cols/matmul back-to-back = the 78.6 TF/s fp16 roofline.
fp16 in/out rounding gives ~8e-4 absmax rel err (f32 psum accumulate).
"""
import numpy as np
from contextlib import ExitStack

import concourse.bass as bass
import concourse.tile as tile
from concourse import bacc, mybir
from concourse.bass import ts, ds
from concourse.bass_utils import run_bass_kernel_spmd

F32 = mybir.dt.float32
F16 = mybir.dt.float16

N_CORES = 8
B, S, DIN, DOUT, R = 8, 2048, 1024, 4096, 8
SCALE = 16.0 / 8
R2 = 2 * R

P = 128            # partition tile
OTILE = 512        # matmul moving free dim (one PSUM bank of f32)
KT = DIN // P      # 8 k-tiles
OT = DOUT // OTILE # 8 o-tiles
ST = S // P        # 16 s-tiles


def build_nc():
    nc = bacc.Bacc("TRN2", target_bir_lowering=False, debug=False,
                   num_devices=N_CORES)
    xT = nc.dram_tensor("xT", [DIN, S], F16, kind="ExternalInput").ap()
    WtT = nc.dram_tensor("WtT", [OT * KT * P, OTILE], F16,
                         kind="ExternalInput").ap()
    bvec = nc.dram_tensor("b16", [DOUT], F16, kind="ExternalInput").ap()
    A_cat = nc.dram_tensor("A_cat", [R2, DIN], F16, kind="ExternalInput").ap()
    B_catT = nc.dram_tensor("B_catT", [R2, DOUT], F16,
                            kind="ExternalInput").ap()
    out = nc.dram_tensor("out", [S, DOUT], F16, kind="ExternalOutput").ap()

    with tile.TileContext(nc) as tc:
        with ExitStack() as ctx:
            const = ctx.enter_context(tc.tile_pool(name="const", bufs=1))
            xt_pool = ctx.enter_context(tc.tile_pool(name="xt", bufs=1))
            wet_pool = ctx.enter_context(tc.tile_pool(name="wet", bufs=3))
            out_pool = ctx.enter_context(tc.tile_pool(name="outp", bufs=4))
            ps_aux = ctx.enter_context(
                tc.tile_pool(name="psaux", bufs=4, space="PSUM"))
            ps_main = ctx.enter_context(
                tc.tile_pool(name="psmain", bufs=4, space="PSUM"))

            # consts; sync queue gets [acat, bcatt, wet...] triggers,
            # scalar queue gets [brow, xt..., out...] triggers (each
            # dma_start costs ~600ns serialized on its trigger queue)
            ones_col = const.tile([1, P], F16)
            nc.vector.memset(ones_col[:], 1.0)
            acat = const.tile([R2, DIN], F16)
            nc.sync.dma_start(acat[:], A_cat)
            bcatt = const.tile([R2, DOUT], F16)
            nc.sync.dma_start(bcatt[:], B_catT)
            brow16 = const.tile([1, DOUT], F16)
            nc.scalar.dma_start(brow16[:], bvec[None, :])
            bias_sb = const.tile([P, DOUT], F32)

            # PE p-state warmup: the PE clock ramps with a slow DVFS governor;
            # short dummy matmuls (dep only on the memset) lift it off the
            # lowest p-state while the first DMAs land. Uses ps_main, which
            # is idle until the first s-loop.
            for i in range(32):
                pw = ps_main.tile([P, OTILE], F32, tag="psmain")
                nc.tensor.matmul(pw[:, :P], ones_col[:], ones_col[:],
                                 start=True, stop=True)

            # W_eff chunks, triple-buffered per kt tag: [128 k, 512 o] fp16.
            # Each build also broadcasts its bias chunk (rank-1 matmul).
            wet = [[None] * KT for _ in range(OT)]

            def build_wet(ot):
                for kt in range(KT):
                    w = wet_pool.tile([P, OTILE], F16, tag=f"wet{kt}",
                                      name=f"wet{ot}_{kt}")
                    nc.sync.dma_start(
                        w[:], WtT[ds((ot * KT + kt) * P, P), :])
                    wet[ot][kt] = w
                for kt in range(KT):
                    pl = ps_aux.tile([P, OTILE], F32, tag="psaux")
                    nc.tensor.matmul(pl[:], acat[:, ts(kt, P)],
                                     bcatt[:, ts(ot, OTILE)],
                                     start=True, stop=True)
                    w = wet[ot][kt]
                    nc.vector.tensor_tensor(w[:], pl[:], w[:],
                                            mybir.AluOpType.add)
                pb = ps_aux.tile([P, OTILE], F32, tag="psaux")
                nc.tensor.matmul(pb[:], ones_col[:],
                                 brow16[:, ts(ot, OTILE)],
                                 start=True, stop=True)
                nc.vector.tensor_copy(bias_sb[:, ts(ot, OTILE)], pb[:])

            build_wet(0)

            # resident x.T: 8 tiles [128 k, 2048 s], 4KB/partition
            xt = []
            for kt in range(KT):
                t = xt_pool.tile([P, S], F16, tag=f"xt{kt}", name=f"xt{kt}")
                nc.scalar.dma_start(t[:], xT[ts(kt, P), :])
                xt.append(t)

            build_wet(1)

            # ---- main: ot-outer, build W_eff[ot+1] ahead of s-loop[ot] ----
            for ot in range(OT):
                if 2 <= ot + 1 < OT:
                    build_wet(ot + 1)
                for st in range(ST):
                    po = ps_main.tile([P, OTILE], F32, tag="psmain")
                    for kt in range(KT):
                        nc.tensor.matmul(po[:], xt[kt][:, ts(st, P)],
                                         wet[ot][kt][:],
                                         start=(kt == 0), stop=(kt == KT - 1))
                    osb = out_pool.tile([P, OTILE], F16)
                    nc.vector.tensor_tensor(osb[:], po[:],
                                            bias_sb[:, ts(ot, OTILE)],
                                            mybir.AluOpType.add)
                    nc.scalar.dma_start(out[ts(st, P), ts(ot, OTILE)], osb[:])

    nc.compile()
    return nc


_NC_CACHE = None


def _get_nc():
    global _NC_CACHE
    if _NC_CACHE is None:
        _NC_CACHE = build_nc()
    return _NC_CACHE


def make_in_maps(x, W, b, global_A, global_B, local_A, local_B):
    x = np.asarray(x, dtype=np.float32)
    W = np.asarray(W, dtype=np.float32)
    b = np.asarray(b, dtype=np.float32)
    # W.T tiled [ot][kt][128, 512] -> [8192, 512] so each o-tile is contiguous
    WtT = np.ascontiguousarray(
        W.T.reshape(KT, P, OT, OTILE).transpose(2, 0, 1, 3)
    ).reshape(OT * KT * P, OTILE).astype(np.float16)
    A_cat = np.ascontiguousarray(
        SCALE * np.concatenate([np.asarray(global_A), np.asarray(local_A)],
                               axis=0)).astype(np.float16)
    B_catT = np.ascontiguousarray(
        np.concatenate([np.asarray(global_B).T, np.asarray(local_B).T],
                       axis=0)).astype(np.float16)
    b16 = b.astype(np.float16)
    return [
        {"xT": np.ascontiguousarray(x[i].T).astype(np.float16),
         "WtT": WtT, "b16": b16, "A_cat": A_cat, "B_catT": B_catT}
        for i in range(N_CORES)
    ]


def kernel(x, W, b, global_A, global_B, local_A, local_B):
    nc = _get_nc()
    in_maps = make_in_maps(x, W, b, global_A, global_B, local_A, local_B)
    res = run_bass_kernel_spmd(nc, in_maps, list(range(N_CORES))).results
    return np.stack([res[i]["out"].astype(np.float32)
                     for i in range(N_CORES)], axis=0)
